# revision 5
# baseline (speedup 1.0000x reference)
"""DeepClusteringLoss on 8 TRN2 NeuronCores.

loss = -sum_b ||E_b^T Y_b||_F^2 / (mean_b ||E_b^T E_b||_F^2 + 1e-8)
with Y = V / (colsum(V) + 1e-8), E: (B, N, D), V: (B, N, S), N = F*T.

Sharding: data-parallel over batch (8 batches -> 8 cores). Each core
reduces its shard to a (120,120) Gram block matrix on-device; the host
sums diagonal blocks and combines the per-batch scalars.

Device algorithm (per core), raw Bass (no Tile framework preamble):
  Host packs each row as 24 fp8e4m3 values [e_0..e_19, v_0, v_1, 1, 0]
  (fp8 quantization of the inputs costs ~2e-3 relative error on the
  final loss, well under the 2e-2 gate; the interleave makes E^T V fall
  out of the same Gram matmul as E^T E, and the embedded ones column
  makes colsum(V) fall out as row 22 of each diagonal block, so there
  is no separate colsum matmul stream).  N=257000 rows are zero-padded
  to 2010*128 = 257280 (pad rows have v=0 so their ones entries are
  inert).  The padded array is split into DMA groups of m_i row-chunks;
  group i is viewed as (128, m_i*24) fp8: partition p holds m_i
  consecutive 24-byte rows, one contiguous DRAM read per partition
  line.  One SWDGE DMA per group (no cast in flight; HBM reads 1/4 of
  the fp32 bytes the baseline moved).
  Matmuls contract over the 128 partitions with the fp8 DoubleRow perf
  mode: a [128, 2, 120] slice (two 5-chunk planes) as both stationary
  and moving accumulates both planes' (120,120) Gram blocks into PSUM
  in a single pass at 2x fp8 column rate.  PSUM-accumulating all slices
  of all groups leaves full-batch [[E^T E, E^T V, colsum^T...]] sums in
  the five diagonal 24x24 blocks.
"""

import sys

if "/opt/trn_rl_repo" not in sys.path:
    sys.path.insert(0, "/opt/trn_rl_repo")

from contextlib import ExitStack

import ml_dtypes
import numpy as np

import concourse.bass as bass
from concourse import mybir
from concourse.bass_utils import run_bass_kernel_spmd

# Problem geometry (hardcoded; see spec)
B, F, T, D, S = 8, 257, 1000, 20, 2
N = F * T  # 257000
CH = 24  # 24 fp8 columns per row: [e0..e19, v0, v1, 1, 0]
P = 128  # SBUF partitions
C = 5  # row-chunks fused per matmul plane (C*CH = 120 <= 128 stationary)
BLK = C * CH  # 120
KP = 2  # DoubleRow k-planes per matmul -> 2*C*CH = 240 fp8 bytes/slice

# Row-chunks per DMA group (sum = 2010 -> NPAD = 257280, 0.1% padding),
# each divisible by C, with the issuing queue per group.  The two HWDGE
# queues (sync=SP, scalar=ACT) have ~300ns first-packet latency and carry
# the early groups so the tensor engine starts at ~4us; SWDGE (gpsimd,
# 16 SDMA engines, ~6us lead-in but ~300 GB/s) carries the bulk,
# arriving by the time the HWDGE groups are consumed.  Groups are
# consumed by the tensor engine in list order.
MS = [25, 50, 100, 125, 150, 150, 240, 480, 480, 160, 40, 10]
MQ = ["sp", "act", "sp", "act", "sp", "act", "sw", "sw", "sw", "sw", "sw", "sw"]
NPAD = P * sum(MS)  # 257280
N_WARM = 3  # PE warm-up dummy matmuls (512 moving cols each)
FP8 = ml_dtypes.float8_e4m3  # matches mybir.dt.np(mybir.dt.float8e4)


def build_bass(ms=None, mq=None, n_cores=B):
    """Build the per-core raw-Bass SPMD program (same program on every
    core; only the input data differs)."""
    ms = list(MS if ms is None else ms)
    mq = list(MQ if mq is None else mq)
    assert all(m % C == 0 for m in ms)
    npad = P * sum(ms)
    ngrp = len(ms)

    nc = bass.Bass("TRN2", debug=False, num_devices=n_cores)
    ev = nc.dram_tensor("ev", [npad, CH], mybir.dt.float8e4, kind="ExternalInput")
    out_g = nc.dram_tensor("out_g", [BLK, BLK], mybir.dt.float32, kind="ExternalOutput")

    # DRAM views per group: (128, m*CH), partition-major rows
    bases = np.cumsum([0] + ms).tolist()
    ev_views = [
        ev.ap()[P * bases[i] : P * bases[i + 1], :].rearrange(
            "(p m) d -> p (m d)", p=P
        )
        for i in range(ngrp)
    ]

    with ExitStack() as ctx:
        bufs = [
            ctx.enter_context(
                nc.sbuf_tensor(f"buf{i}", [P, m * CH], mybir.dt.float8e4)
            )
            for i, m in enumerate(ms)
        ]
        scr = ctx.enter_context(nc.sbuf_tensor("scr", [P, 512], mybir.dt.float8e4))
        gsb = ctx.enter_context(nc.sbuf_tensor("gsb", [BLK, BLK], mybir.dt.float32))
        gacc = ctx.enter_context(nc.psum_tensor("gacc", [P, BLK], mybir.dt.float32))
        warm_ps = ctx.enter_context(
            nc.psum_tensor("warm_ps", [P, 512], mybir.dt.float32)
        )
        dma_sems = [
            ctx.enter_context(nc.semaphore(f"dma_sem{i}")) for i in range(ngrp)
        ]
        warm_sem = ctx.enter_context(nc.semaphore("warm_sem"))
        ten_sem = ctx.enter_context(nc.semaphore("ten_sem"))
        copy_sem = ctx.enter_context(nc.semaphore("copy_sem"))
        odma_sem = ctx.enter_context(nc.semaphore("odma_sem"))
        block = ctx.enter_context(nc.Block(no_gpsimd_drain=True))

        def issue_dmas(eng, queue):
            for i in range(ngrp):
                if mq[i] == queue:
                    # One semaphore per group: a sem with a single DMA in
                    # flight reads 16 exactly when that DMA fully landed.
                    eng.dma_start(out=bufs[i].ap(), in_=ev_views[i]).then_inc(
                        dma_sems[i], 16
                    )

        @block.gpsimd
        def _(g: bass.BassEngine):
            issue_dmas(g, "sw")

        @block.tensor
        def _(t: bass.BassEngine):
            # PE warm-up: dummy matmuls on scratch data so the 3us
            # continuous-execution clock ramp (1.2 -> 2.4 GHz) overlaps
            # the DMA wait instead of the real matmul stream.
            t.wait_ge(warm_sem, 1)
            for _ in range(N_WARM):
                t.matmul(
                    warm_ps.ap(), scr.ap()[:, :P], scr.ap(), start=True, stop=True
                )
            # Plain fp8 matmuls (1 moving col/cycle).  DoubleRow is a net
            # loss for self-Gram: stationary==moving, so its 2x stream win
            # is cancelled by the doubled LDWEIGHTS (256-col load).
            # Stationary takes 128 columns (5 chunks + 8 slack bytes of the
            # next slice) to hit the compiler's NumWeights==128 FWL trigger
            # (4-bytes-per-cycle weight load); PSUM rows 120..127 are
            # garbage and ignored.  The last slice of each group has no
            # slack to borrow, so it stays at 120 columns.
            total = sum(m // C for m in ms)
            gi = 0
            for i, m in enumerate(ms):
                t.wait_ge(dma_sems[i], 16)
                buf = bufs[i]
                for j in range(m // C):
                    mov = buf.ap()[:, j * BLK : (j + 1) * BLK]
                    wcols = P if (j * BLK + P) <= m * CH else BLK
                    sta = buf.ap()[:, j * BLK : j * BLK + wcols]
                    last = t.matmul(
                        gacc.ap()[:wcols, :],
                        sta,
                        mov,
                        start=(gi == 0),
                        stop=(gi == total - 1),
                    )
                    gi += 1
            last.then_inc(ten_sem, 1)

        @block.vector
        def _(v: bass.BassEngine):
            v.memset(scr.ap(), 0.03125).then_inc(warm_sem, 1)
            # DVE does the PSUM -> SBUF copy (ACT would pay a ~1.3 us
            # activation-table load for its first ACTIVATE)
            v.wait_ge(ten_sem, 1)
            v.tensor_copy(gsb.ap(), gacc.ap()[:BLK, :]).then_inc(copy_sem, 1)

        @block.scalar
        def _(sc: bass.BassEngine):
            issue_dmas(sc, "act")

        @block.sync
        def _(s: bass.BassEngine):
            issue_dmas(s, "sp")
            s.wait_ge(copy_sem, 1)
            s.dma_start(out=out_g.ap(), in_=gsb.ap()).then_inc(odma_sem, 16)
            s.wait_ge(odma_sem, 16)

    return nc


def pack_inputs(embeddings, source_indicators, npad=NPAD):
    """(B,F,T,D)+(B,F,T,S) -> per-core padded interleaved (npad, 24) fp8."""
    b = embeddings.shape[0]
    n = embeddings.shape[1] * embeddings.shape[2]
    e = np.asarray(embeddings, dtype=np.float32).reshape(b, n, D)
    v = np.asarray(source_indicators, dtype=np.float32).reshape(b, n, S)
    evp = np.zeros((b, npad, CH), dtype=FP8)
    evp[:, :n, :D] = e.astype(FP8)
    evp[:, :n, D : D + S] = v.astype(FP8)
    evp[:, :, D + S] = np.asarray(1.0, dtype=FP8)
    return evp


def reduce_outputs(res):
    """Per-core raw output -> (G_b, EtV_b, colsum_b) in float64."""
    out_g = np.asarray(res["out_g"], dtype=np.float64)
    g_b = np.zeros((D, D))
    etv_b = np.zeros((D, S))
    colsum_b = np.zeros(S)
    for c in range(C):
        blk = out_g[c * CH : (c + 1) * CH, c * CH : (c + 1) * CH]
        g_b += blk[:D, :D]
        etv_b += blk[:D, D : D + S]
        colsum_b += blk[D + S, D : D + S]
    return g_b, etv_b, colsum_b


_NC_CACHE = {}


def _get_nc():
    if "nc" not in _NC_CACHE:
        _NC_CACHE["nc"] = build_bass()
    return _NC_CACHE["nc"]


def kernel(embeddings, source_indicators):
    evp = pack_inputs(embeddings, source_indicators)
    nc = _get_nc()
    in_maps = [{"ev": np.ascontiguousarray(evp[b])} for b in range(B)]
    results = run_bass_kernel_spmd(nc, in_maps, list(range(B))).results

    loss = 0.0
    norms = []
    for b in range(B):
        g_b, etv_b, colsum_b = reduce_outputs(results[b])
        ety = etv_b / (colsum_b[None, :] + 1e-8)
        loss += float(np.sum(ety * ety))
        norms.append(float(np.sum(g_b * g_b)))
    norm_term = float(np.mean(norms))
    return np.float32(-loss / (norm_term + 1e-8))


# revision 6
# speedup vs baseline: 1.1810x; 1.1810x over previous
"""DeepClusteringLoss on 8 TRN2 NeuronCores.

loss = -sum_b ||E_b^T Y_b||_F^2 / (mean_b ||E_b^T E_b||_F^2 + 1e-8)
with Y = V / (colsum(V) + 1e-8), E: (B, N, D), V: (B, N, S), N = F*T.

Sharding: data-parallel over batch (8 batches -> 8 cores). Each core
reduces its shard to a (120,120) Gram block matrix on-device; the host
sums diagonal blocks and combines the per-batch scalars.

Device algorithm (per core), raw Bass (no Tile framework preamble):
  Host packs each row as 24 fp8e4m3 values [e_0..e_19, v_0, v_1, 1, 0]
  (fp8 quantization of the inputs costs ~2e-3 relative error on the
  final loss, well under the 2e-2 gate; the interleave makes E^T V fall
  out of the same Gram matmul as E^T E, and the embedded ones column
  makes colsum(V) fall out as row 22 of each diagonal block, so there
  is no separate colsum matmul stream).  N=257000 rows are zero-padded
  to 2010*128 = 257280 (pad rows have v=0 so their ones entries are
  inert).  The padded array is split into DMA groups of m_i row-chunks;
  group i is viewed as (128, m_i*24) fp8: partition p holds m_i
  consecutive 24-byte rows, one contiguous DRAM read per partition
  line.  One SWDGE DMA per group (no cast in flight; HBM reads 1/4 of
  the fp32 bytes the baseline moved).
  Matmuls contract over the 128 partitions with the fp8 DoubleRow perf
  mode: a [128, 2, 120] slice (two 5-chunk planes) as both stationary
  and moving accumulates both planes' (120,120) Gram blocks into PSUM
  in a single pass at 2x fp8 column rate.  PSUM-accumulating all slices
  of all groups leaves full-batch [[E^T E, E^T V, colsum^T...]] sums in
  the five diagonal 24x24 blocks.
"""

import sys

if "/opt/trn_rl_repo" not in sys.path:
    sys.path.insert(0, "/opt/trn_rl_repo")

from contextlib import ExitStack

import ml_dtypes
import numpy as np

import concourse.bass as bass
from concourse import mybir
from concourse.bass_utils import run_bass_kernel_spmd

# Problem geometry (hardcoded; see spec)
B, F, T, D, S = 8, 257, 1000, 20, 2
N = F * T  # 257000
CH = 24  # 24 fp8 columns per row: [e0..e19, v0, v1, 1, 0]
P = 128  # SBUF partitions
C = 5  # row-chunks fused per matmul plane (C*CH = 120 <= 128 stationary)
BLK = C * CH  # 120
KP = 2  # DoubleRow k-planes per matmul -> 2*C*CH = 240 fp8 bytes/slice

# Row-chunks per DMA group (sum = 2010 -> NPAD = 257280, 0.1% padding),
# each divisible by C.  All groups go through SWDGE (gpsimd; 16 SDMA
# engines, ~300 GB/s, first packet ~9.5us after the fixed ~6us engine
# preamble).  HWDGE (sync/scalar) was measured at only ~55 GB/s per
# queue with its own ~5us lead-in -- useless for input staging.  Group
# sizes ramp up so each group lands just as the tensor engine (cold
# 1.2 GHz -> warm 2.4 GHz, ~300 GB/s warm) finishes the previous one,
# then decay so the stream and the matmul backlog end together.
MS = [60, 120, 180, 240, 360, 480, 360, 140, 50, 20]
MQ = ["sw"] * len(MS)
NPAD = P * sum(MS)  # 257280
N_WARM = 8  # PE warm-up dummy matmuls (512 moving cols each)
FP8 = ml_dtypes.float8_e4m3  # matches mybir.dt.np(mybir.dt.float8e4)


def build_bass(ms=None, mq=None, n_cores=B):
    """Build the per-core raw-Bass SPMD program (same program on every
    core; only the input data differs)."""
    ms = list(MS if ms is None else ms)
    mq = list(MQ if mq is None else mq)
    assert all(m % C == 0 for m in ms)
    npad = P * sum(ms)
    ngrp = len(ms)

    nc = bass.Bass("TRN2", debug=False, num_devices=n_cores)
    ev = nc.dram_tensor("ev", [npad, CH], mybir.dt.float8e4, kind="ExternalInput")
    out_g = nc.dram_tensor("out_g", [BLK, BLK], mybir.dt.float32, kind="ExternalOutput")

    # DRAM views per group: (128, m*CH), partition-major rows
    bases = np.cumsum([0] + ms).tolist()
    ev_views = [
        ev.ap()[P * bases[i] : P * bases[i + 1], :].rearrange(
            "(p m) d -> p (m d)", p=P
        )
        for i in range(ngrp)
    ]

    with ExitStack() as ctx:
        bufs = [
            ctx.enter_context(
                nc.sbuf_tensor(f"buf{i}", [P, m * CH], mybir.dt.float8e4)
            )
            for i, m in enumerate(ms)
        ]
        scr = ctx.enter_context(nc.sbuf_tensor("scr", [P, 512], mybir.dt.float8e4))
        gsb = ctx.enter_context(nc.sbuf_tensor("gsb", [BLK, BLK], mybir.dt.float32))
        gacc = ctx.enter_context(nc.psum_tensor("gacc", [P, BLK], mybir.dt.float32))
        warm_ps = ctx.enter_context(
            nc.psum_tensor("warm_ps", [P, 512], mybir.dt.float32)
        )
        dma_sems = [
            ctx.enter_context(nc.semaphore(f"dma_sem{i}")) for i in range(ngrp)
        ]
        warm_sem = ctx.enter_context(nc.semaphore("warm_sem"))
        ten_sem = ctx.enter_context(nc.semaphore("ten_sem"))
        copy_sem = ctx.enter_context(nc.semaphore("copy_sem"))
        odma_sem = ctx.enter_context(nc.semaphore("odma_sem"))
        block = ctx.enter_context(nc.Block(no_gpsimd_drain=True))

        def issue_dmas(eng, queue):
            for i in range(ngrp):
                if mq[i] == queue:
                    # One semaphore per group: a sem with a single DMA in
                    # flight reads 16 exactly when that DMA fully landed.
                    eng.dma_start(out=bufs[i].ap(), in_=ev_views[i]).then_inc(
                        dma_sems[i], 16
                    )

        @block.gpsimd
        def _(g: bass.BassEngine):
            issue_dmas(g, "sw")

        @block.tensor
        def _(t: bass.BassEngine):
            # PE warm-up: dummy matmuls on scratch data so the 3us
            # continuous-execution clock ramp (1.2 -> 2.4 GHz) overlaps
            # the DMA wait instead of the real matmul stream.
            t.wait_ge(warm_sem, 1)
            for _ in range(N_WARM):
                t.matmul(
                    warm_ps.ap(), scr.ap()[:, :P], scr.ap(), start=True, stop=True
                )
            # Plain fp8 matmuls (1 moving col/cycle).  DoubleRow is a net
            # loss for self-Gram: stationary==moving, so its 2x stream win
            # is cancelled by the doubled LDWEIGHTS (256-col load).
            # Stationary takes 128 columns (5 chunks + 8 slack bytes of the
            # next slice) to hit the compiler's NumWeights==128 FWL trigger
            # (4-bytes-per-cycle weight load); PSUM rows 120..127 are
            # garbage and ignored.  The last slice of each group has no
            # slack to borrow, so it stays at 120 columns.
            total = sum(m // C for m in ms)
            gi = 0
            for i, m in enumerate(ms):
                t.wait_ge(dma_sems[i], 16)
                buf = bufs[i]
                for j in range(m // C):
                    mov = buf.ap()[:, j * BLK : (j + 1) * BLK]
                    wcols = P if (j * BLK + P) <= m * CH else BLK
                    sta = buf.ap()[:, j * BLK : j * BLK + wcols]
                    last = t.matmul(
                        gacc.ap()[:wcols, :],
                        sta,
                        mov,
                        start=(gi == 0),
                        stop=(gi == total - 1),
                    )
                    gi += 1
            last.then_inc(ten_sem, 1)

        @block.vector
        def _(v: bass.BassEngine):
            v.memset(scr.ap(), 0.03125).then_inc(warm_sem, 1)
            # DVE does the PSUM -> SBUF copy (ACT would pay a ~1.3 us
            # activation-table load for its first ACTIVATE)
            v.wait_ge(ten_sem, 1)
            v.tensor_copy(gsb.ap(), gacc.ap()[:BLK, :]).then_inc(copy_sem, 1)

        @block.scalar
        def _(sc: bass.BassEngine):
            issue_dmas(sc, "act")

        @block.sync
        def _(s: bass.BassEngine):
            issue_dmas(s, "sp")
            s.wait_ge(copy_sem, 1)
            s.dma_start(out=out_g.ap(), in_=gsb.ap()).then_inc(odma_sem, 16)
            s.wait_ge(odma_sem, 16)

    return nc


def pack_inputs(embeddings, source_indicators, npad=NPAD):
    """(B,F,T,D)+(B,F,T,S) -> per-core padded interleaved (npad, 24) fp8."""
    b = embeddings.shape[0]
    n = embeddings.shape[1] * embeddings.shape[2]
    e = np.asarray(embeddings, dtype=np.float32).reshape(b, n, D)
    v = np.asarray(source_indicators, dtype=np.float32).reshape(b, n, S)
    evp = np.zeros((b, npad, CH), dtype=FP8)
    evp[:, :n, :D] = e.astype(FP8)
    evp[:, :n, D : D + S] = v.astype(FP8)
    evp[:, :, D + S] = np.asarray(1.0, dtype=FP8)
    return evp


def reduce_outputs(res):
    """Per-core raw output -> (G_b, EtV_b, colsum_b) in float64."""
    out_g = np.asarray(res["out_g"], dtype=np.float64)
    g_b = np.zeros((D, D))
    etv_b = np.zeros((D, S))
    colsum_b = np.zeros(S)
    for c in range(C):
        blk = out_g[c * CH : (c + 1) * CH, c * CH : (c + 1) * CH]
        g_b += blk[:D, :D]
        etv_b += blk[:D, D : D + S]
        colsum_b += blk[D + S, D : D + S]
    return g_b, etv_b, colsum_b


_NC_CACHE = {}


def _get_nc():
    if "nc" not in _NC_CACHE:
        _NC_CACHE["nc"] = build_bass()
    return _NC_CACHE["nc"]


def kernel(embeddings, source_indicators):
    evp = pack_inputs(embeddings, source_indicators)
    nc = _get_nc()
    in_maps = [{"ev": np.ascontiguousarray(evp[b])} for b in range(B)]
    results = run_bass_kernel_spmd(nc, in_maps, list(range(B))).results

    loss = 0.0
    norms = []
    for b in range(B):
        g_b, etv_b, colsum_b = reduce_outputs(results[b])
        ety = etv_b / (colsum_b[None, :] + 1e-8)
        loss += float(np.sum(ety * ety))
        norms.append(float(np.sum(g_b * g_b)))
    norm_term = float(np.mean(norms))
    return np.float32(-loss / (norm_term + 1e-8))


# revision 17
# speedup vs baseline: 1.1823x; 1.0011x over previous
"""DeepClusteringLoss on 8 TRN2 NeuronCores.

loss = -sum_b ||E_b^T Y_b||_F^2 / (mean_b ||E_b^T E_b||_F^2 + 1e-8)
with Y = V / (colsum(V) + 1e-8), E: (B, N, D), V: (B, N, S), N = F*T.

Sharding: data-parallel over batch (8 batches -> 8 cores). Each core
reduces its shard to a (120,120) Gram block matrix on-device; the host
sums diagonal blocks and combines the per-batch scalars.

Device algorithm (per core), raw Bass (no Tile framework preamble):
  Host packs each row as 24 fp8e4m3 values [e_0..e_19, v_0, v_1, 1, 0]
  (fp8 quantization of the inputs costs ~2e-3 relative error on the
  final loss, well under the 2e-2 gate; the interleave makes E^T V fall
  out of the same Gram matmul as E^T E, and the embedded ones column
  makes colsum(V) fall out as row 22 of each diagonal block, so there
  is no separate colsum matmul stream).  N=257000 rows are zero-padded
  to 2010*128 = 257280 (pad rows have v=0 so their ones entries are
  inert).  The padded array is split into DMA groups of m_i row-chunks;
  group i is viewed as (128, m_i*24) fp8: partition p holds m_i
  consecutive 24-byte rows, one contiguous DRAM read per partition
  line.  One SWDGE DMA per group (no cast in flight; HBM reads 1/4 of
  the fp32 bytes the baseline moved).
  Matmuls contract over the 128 partitions with the fp8 DoubleRow perf
  mode: a [128, 2, 120] slice (two 5-chunk planes) as both stationary
  and moving accumulates both planes' (120,120) Gram blocks into PSUM
  in a single pass at 2x fp8 column rate.  PSUM-accumulating all slices
  of all groups leaves full-batch [[E^T E, E^T V, colsum^T...]] sums in
  the five diagonal 24x24 blocks.
"""

import sys

if "/opt/trn_rl_repo" not in sys.path:
    sys.path.insert(0, "/opt/trn_rl_repo")

from contextlib import ExitStack

import ml_dtypes
import numpy as np

import concourse.bass as bass
from concourse import mybir
from concourse.bass_utils import run_bass_kernel_spmd

# Problem geometry (hardcoded; see spec)
B, F, T, D, S = 8, 257, 1000, 20, 2
N = F * T  # 257000
CH = 24  # 24 fp8 columns per row: [e0..e19, v0, v1, 1, 0]
P = 128  # SBUF partitions
C = 5  # row-chunks fused per matmul plane (C*CH = 120 <= 128 stationary)
BLK = C * CH  # 120
KP = 2  # DoubleRow k-planes per matmul -> 2*C*CH = 240 fp8 bytes/slice

# Row-chunks per DMA group (sum = 2010 -> NPAD = 257280, 0.1% padding),
# each divisible by C.  All groups go through SWDGE (gpsimd; 16 SDMA
# engines, ~300 GB/s, first packet ~9.5us after the fixed ~6us engine
# preamble).  HWDGE (sync/scalar) was measured at only ~55 GB/s per
# queue with its own ~5us lead-in -- useless for input staging.  Group
# sizes ramp up so each group lands just as the tensor engine (cold
# 1.2 GHz -> warm 2.4 GHz, ~300 GB/s warm) finishes the previous one,
# then decay so the stream and the matmul backlog end together.
MS = [30, 90, 180, 360, 480, 480, 240, 100, 40, 10]
MQ = ["sw"] * len(MS)
NPAD = P * sum(MS)  # 257280
N_WARM = 8  # PE warm-up dummy matmuls (512 moving cols each)
MCOL = CH - 1  # moving cols per chunk: stream skips the dead pad column
FP8 = ml_dtypes.float8_e4m3  # matches mybir.dt.np(mybir.dt.float8e4)


def build_bass(ms=None, mq=None, n_cores=B):
    """Build the per-core raw-Bass SPMD program (same program on every
    core; only the input data differs)."""
    ms = list(MS if ms is None else ms)
    mq = list(MQ if mq is None else mq)
    assert all(m % C == 0 for m in ms)
    npad = P * sum(ms)
    ngrp = len(ms)

    nc = bass.Bass("TRN2", debug=False, num_devices=n_cores)
    ev = nc.dram_tensor("ev", [npad, CH], mybir.dt.float8e4, kind="ExternalInput")
    out_g = nc.dram_tensor(
        "out_g", [BLK, C * MCOL], mybir.dt.float32, kind="ExternalOutput"
    )

    # DRAM views per group: (128, m*CH), partition-major rows
    bases = np.cumsum([0] + ms).tolist()
    ev_views = [
        ev.ap()[P * bases[i] : P * bases[i + 1], :].rearrange(
            "(p m) d -> p (m d)", p=P
        )
        for i in range(ngrp)
    ]

    with ExitStack() as ctx:
        bufs = [
            ctx.enter_context(
                nc.sbuf_tensor(f"buf{i}", [P, m * CH], mybir.dt.float8e4)
            )
            for i, m in enumerate(ms)
        ]
        scr = ctx.enter_context(nc.sbuf_tensor("scr", [P, 512], mybir.dt.float8e4))
        gsb = ctx.enter_context(
            nc.sbuf_tensor("gsb", [BLK, C * MCOL], mybir.dt.float32)
        )
        gsb2 = ctx.enter_context(
            nc.sbuf_tensor("gsb2", [BLK, C * MCOL], mybir.dt.float32)
        )
        # Two accumulator banks: consecutive matmuls alternate banks so the
        # fill of matmul i+1 is not RAW-serialized on the PSUM drain of
        # matmul i (same-address accumulation forces drain-to-fill spacing;
        # PSUM allocation is bank-granular, so these decouple fully).
        gaccs = [
            ctx.enter_context(nc.psum_tensor(f"gacc{k}", [P, C * MCOL], mybir.dt.float32))
            for k in range(2)
        ]
        warm_ps = ctx.enter_context(
            nc.psum_tensor("warm_ps", [P, 512], mybir.dt.float32)
        )
        dma_sems = [
            ctx.enter_context(nc.semaphore(f"dma_sem{i}")) for i in range(ngrp)
        ]
        warm_sem = ctx.enter_context(nc.semaphore("warm_sem"))
        stage_sem = ctx.enter_context(nc.semaphore("stage_sem"))
        ten_sem = ctx.enter_context(nc.semaphore("ten_sem"))
        copy_sem = ctx.enter_context(nc.semaphore("copy_sem"))
        odma_sem = ctx.enter_context(nc.semaphore("odma_sem"))
        block = ctx.enter_context(nc.Block(no_gpsimd_drain=True))

        def issue_dmas(eng, queue):
            for i in range(ngrp):
                if mq[i] == queue:
                    # One semaphore per group: a sem with a single DMA in
                    # flight reads 16 exactly when that DMA fully landed.
                    eng.dma_start(out=bufs[i].ap(), in_=ev_views[i]).then_inc(
                        dma_sems[i], 16
                    )

        @block.gpsimd
        def _(g: bass.BassEngine):
            issue_dmas(g, "sw")

        @block.tensor
        def _(t: bass.BassEngine):
            # PE warm-up: dummy matmuls on scratch data so the 3us
            # continuous-execution clock ramp (1.2 -> 2.4 GHz) overlaps
            # the DMA wait instead of the real matmul stream.
            t.wait_ge(warm_sem, 1)
            for _ in range(N_WARM):
                t.matmul(
                    warm_ps.ap(), scr.ap()[:, :P], scr.ap(), start=True, stop=True
                )
            # Plain fp8 matmuls (1 moving col/cycle).  DoubleRow is a net
            # loss for self-Gram: stationary==moving, so its 2x stream win
            # is cancelled by the doubled LDWEIGHTS (256-col load).
            # Stationary takes 128 columns (5 chunks + 8 slack bytes of the
            # next slice) to hit the compiler's NumWeights==128 FWL trigger
            # (4-bytes-per-cycle weight load); PSUM rows 120..127 are
            # garbage and ignored.  The last slice of each group has no
            # slack to borrow, so it stays at 120 columns.
            total = sum(m // C for m in ms)
            gi = 0
            for i, m in enumerate(ms):
                t.wait_ge(dma_sems[i], 16)
                buf = bufs[i]
                bview = buf.ap().rearrange("p (m d) -> p m d", d=CH)
                for j in range(m // C):
                    # moving skips the pad column: [128, C, 23] -> 115 cols
                    mov = bview[:, j * C : (j + 1) * C, :MCOL]
                    wcols = P if (j * BLK + P) <= m * CH else BLK
                    sta = buf.ap()[:, j * BLK : j * BLK + wcols]
                    last = t.matmul(
                        gaccs[gi % 2].ap()[:wcols, :],
                        sta,
                        mov,
                        start=(gi < 2),
                        stop=(gi >= total - 2),
                    )
                    gi += 1
            last.then_inc(ten_sem, 1)

        @block.vector
        def _(v: bass.BassEngine):
            v.memset(scr.ap(), 0.03125).then_inc(warm_sem, 1)
            # DVE sums the two PSUM banks straight into SBUF (ACT would pay
            # a ~1.3 us activation-table load for its first ACTIVATE)
            v.wait_ge(ten_sem, 1)
            # walrus: only one non-scalar input may come from PSUM per op
            v.tensor_copy(gsb2.ap(), gaccs[0].ap()[:BLK, :]).then_inc(stage_sem, 1)
            v.wait_ge(stage_sem, 1)
            v.scalar_tensor_tensor(
                gsb.ap(),
                gaccs[1].ap()[:BLK, :],
                1.0,
                gsb2.ap(),
                op0=mybir.AluOpType.mult,
                op1=mybir.AluOpType.add,
            ).then_inc(copy_sem, 1)

        @block.scalar
        def _(sc: bass.BassEngine):
            issue_dmas(sc, "act")

        @block.sync
        def _(s: bass.BassEngine):
            issue_dmas(s, "sp")
            s.wait_ge(copy_sem, 1)
            s.dma_start(out=out_g.ap(), in_=gsb.ap()).then_inc(odma_sem, 16)
            s.wait_ge(odma_sem, 16)

    return nc


def pack_inputs(embeddings, source_indicators, npad=NPAD):
    """(B,F,T,D)+(B,F,T,S) -> per-core padded interleaved (npad, 24) fp8."""
    b = embeddings.shape[0]
    n = embeddings.shape[1] * embeddings.shape[2]
    e = np.asarray(embeddings, dtype=np.float32).reshape(b, n, D)
    v = np.asarray(source_indicators, dtype=np.float32).reshape(b, n, S)
    evp = np.zeros((b, npad, CH), dtype=FP8)
    evp[:, :n, :D] = e.astype(FP8)
    evp[:, :n, D : D + S] = v.astype(FP8)
    evp[:, :, D + S] = np.asarray(1.0, dtype=FP8)
    return evp


def reduce_outputs(res):
    """Per-core raw output -> (G_b, EtV_b, colsum_b) in float64.

    Output rows follow the 24-col stationary chunks, output columns the
    23-col (pad-skipped) moving chunks."""
    out_g = np.asarray(res["out_g"], dtype=np.float64)
    g_b = np.zeros((D, D))
    etv_b = np.zeros((D, S))
    colsum_b = np.zeros(S)
    for c in range(C):
        blk = out_g[c * CH : c * CH + CH, c * MCOL : c * MCOL + MCOL]
        g_b += blk[:D, :D]
        etv_b += blk[:D, D : D + S]
        colsum_b += blk[D + S, D : D + S]
    return g_b, etv_b, colsum_b


_NC_CACHE = {}


def _get_nc():
    if "nc" not in _NC_CACHE:
        _NC_CACHE["nc"] = build_bass()
    return _NC_CACHE["nc"]


def kernel(embeddings, source_indicators):
    evp = pack_inputs(embeddings, source_indicators)
    nc = _get_nc()
    in_maps = [{"ev": np.ascontiguousarray(evp[b])} for b in range(B)]
    results = run_bass_kernel_spmd(nc, in_maps, list(range(B))).results

    loss = 0.0
    norms = []
    for b in range(B):
        g_b, etv_b, colsum_b = reduce_outputs(results[b])
        ety = etv_b / (colsum_b[None, :] + 1e-8)
        loss += float(np.sum(ety * ety))
        norms.append(float(np.sum(g_b * g_b)))
    norm_term = float(np.mean(norms))
    return np.float32(-loss / (norm_term + 1e-8))


# revision 29
# speedup vs baseline: 1.1962x; 1.0117x over previous
"""DeepClusteringLoss on 8 TRN2 NeuronCores.

loss = -sum_b ||E_b^T Y_b||_F^2 / (mean_b ||E_b^T E_b||_F^2 + 1e-8)
with Y = V / (colsum(V) + 1e-8), E: (B, N, D), V: (B, N, S), N = F*T.

Sharding: data-parallel over batch (8 batches -> 8 cores). Each core
reduces its shard to a (120,120) Gram block matrix on-device; the host
sums diagonal blocks and combines the per-batch scalars.

Device algorithm (per core), raw Bass (no Tile framework preamble):
  Host packs each row as 24 fp8e4m3 values [e_0..e_19, v_0, v_1, 1, 0]
  (fp8 quantization of the inputs costs ~2e-3 relative error on the
  final loss, well under the 2e-2 gate; the interleave makes E^T V fall
  out of the same Gram matmul as E^T E, and the embedded ones column
  makes colsum(V) fall out as row 22 of each diagonal block, so there
  is no separate colsum matmul stream).  N=257000 rows are zero-padded
  to 2010*128 = 257280 (pad rows have v=0 so their ones entries are
  inert).  The padded array is split into DMA groups of m_i row-chunks;
  group i is viewed as (128, m_i*24) fp8: partition p holds m_i
  consecutive 24-byte rows, one contiguous DRAM read per partition
  line.  One SWDGE DMA per group (no cast in flight; HBM reads 1/4 of
  the fp32 bytes the baseline moved).
  Matmuls contract over the 128 partitions with the fp8 DoubleRow perf
  mode: a [128, 2, 120] slice (two 5-chunk planes) as both stationary
  and moving accumulates both planes' (120,120) Gram blocks into PSUM
  in a single pass at 2x fp8 column rate.  PSUM-accumulating all slices
  of all groups leaves full-batch [[E^T E, E^T V, colsum^T...]] sums in
  the five diagonal 24x24 blocks.
"""

import sys

if "/opt/trn_rl_repo" not in sys.path:
    sys.path.insert(0, "/opt/trn_rl_repo")

from contextlib import ExitStack

import ml_dtypes
import numpy as np

import concourse.bass as bass
from concourse import mybir
from concourse.bass_utils import run_bass_kernel_spmd

# Problem geometry (hardcoded; see spec)
B, F, T, D, S = 8, 257, 1000, 20, 2
N = F * T  # 257000
CH = 24  # 24 fp8 columns per row: [e0..e19, v0, v1, 1, 0]
P = 128  # SBUF partitions
C = 5  # row-chunks fused per matmul plane (C*CH = 120 <= 128 stationary)
BLK = C * CH  # 120
KP = 2  # DoubleRow k-planes per matmul -> 2*C*CH = 240 fp8 bytes/slice

# Row-chunks per DMA group (sum = 2010 -> NPAD = 257280, 0.1% padding),
# each divisible by C.  All groups go through SWDGE (gpsimd; 16 SDMA
# engines, ~300 GB/s, first packet ~9.5us after the fixed ~6us engine
# preamble).  HWDGE (sync/scalar) was measured at only ~55 GB/s per
# queue with its own ~5us lead-in -- useless for input staging.  Group
# sizes ramp up so each group lands just as the tensor engine (cold
# 1.2 GHz -> warm 2.4 GHz, ~300 GB/s warm) finishes the previous one,
# then decay so the stream and the matmul backlog end together.
MS = [30, 90, 180, 360, 480, 480, 240, 100, 40, 10]
MQ = ["sw"] * len(MS)
NPAD = P * sum(MS)  # 257280
N_WARM = 3  # PE warm-up dummy matmuls (512 moving cols each)
MCOL = CH - 1  # moving cols per chunk: stream skips the dead pad column
FP8 = ml_dtypes.float8_e4m3  # matches mybir.dt.np(mybir.dt.float8e4)


def build_bass(ms=None, mq=None, n_cores=B):
    """Build the per-core raw-Bass SPMD program (same program on every
    core; only the input data differs)."""
    ms = list(MS if ms is None else ms)
    mq = list(MQ if mq is None else mq)
    assert all(m % C == 0 for m in ms)
    npad = P * sum(ms)
    ngrp = len(ms)

    nc = bass.Bass("TRN2", debug=False, num_devices=n_cores)
    ev = nc.dram_tensor("ev", [npad, CH], mybir.dt.float8e4, kind="ExternalInput")
    out_g = nc.dram_tensor(
        "out_g", [BLK, C * MCOL], mybir.dt.float32, kind="ExternalOutput"
    )

    # DRAM views per group: (128, m*CH), partition-major rows
    bases = np.cumsum([0] + ms).tolist()
    ev_views = [
        ev.ap()[P * bases[i] : P * bases[i + 1], :].rearrange(
            "(p m) d -> p (m d)", p=P
        )
        for i in range(ngrp)
    ]

    with ExitStack() as ctx:
        bufs = [
            ctx.enter_context(
                nc.sbuf_tensor(f"buf{i}", [P, m * CH], mybir.dt.float8e4)
            )
            for i, m in enumerate(ms)
        ]
        scr = ctx.enter_context(nc.sbuf_tensor("scr", [P, 512], mybir.dt.float8e4))
        gsb = ctx.enter_context(
            nc.sbuf_tensor("gsb", [BLK, C * MCOL], mybir.dt.float32)
        )
        gacc = ctx.enter_context(
            nc.psum_tensor("gacc", [P, C * MCOL], mybir.dt.float32)
        )
        warm_ps = ctx.enter_context(
            nc.psum_tensor("warm_ps", [P, 512], mybir.dt.float32)
        )
        dma_sems = [
            ctx.enter_context(nc.semaphore(f"dma_sem{i}")) for i in range(ngrp)
        ]
        warm_sem = ctx.enter_context(nc.semaphore("warm_sem"))
        ten_sem = ctx.enter_context(nc.semaphore("ten_sem"))
        copy_sem = ctx.enter_context(nc.semaphore("copy_sem"))
        odma_sem = ctx.enter_context(nc.semaphore("odma_sem"))
        block = ctx.enter_context(nc.Block(no_gpsimd_drain=True))

        def issue_dmas(eng, queue):
            for i in range(ngrp):
                if mq[i] == queue:
                    # One semaphore per group: a sem with a single DMA in
                    # flight reads 16 exactly when that DMA fully landed.
                    eng.dma_start(out=bufs[i].ap(), in_=ev_views[i]).then_inc(
                        dma_sems[i], 16
                    )

        @block.gpsimd
        def _(g: bass.BassEngine):
            issue_dmas(g, "sw")

        @block.tensor
        def _(t: bass.BassEngine):
            # PE warm-up: dummy matmuls on scratch so the clock ramp
            # (1.2 -> 2.4 GHz after ~3us continuous execution) overlaps the
            # DMA wait instead of the real matmul stream.  warm_ps is never
            # read.  Sized to end as group 0 lands (~9.4us).
            t.wait_ge(warm_sem, 1)
            for _ in range(N_WARM):
                t.matmul(
                    warm_ps.ap(), scr.ap()[:, :P], scr.ap(), start=True, stop=True
                )
            # Plain fp8 matmuls (1 moving col/cycle).  DoubleRow is a net
            # loss for self-Gram: stationary==moving, so its 2x stream win
            # is cancelled by the doubled LDWEIGHTS (256-col load).
            # Stationary takes 128 columns (5 chunks + 8 slack bytes of the
            # next slice) to hit the compiler's NumWeights==128 FWL trigger
            # (4-bytes-per-cycle weight load); PSUM rows 120..127 are
            # garbage and ignored.  The last slice of each group has no
            # slack to borrow, so it stays at 120 columns.
            total = sum(m // C for m in ms)
            gi = 0
            for i, m in enumerate(ms):
                t.wait_ge(dma_sems[i], 16)
                buf = bufs[i]
                bview = buf.ap().rearrange("p (m d) -> p m d", d=CH)
                for j in range(m // C):
                    # moving skips the pad column: [128, C, 23] -> 115 cols
                    mov = bview[:, j * C : (j + 1) * C, :MCOL]
                    wcols = P if (j * BLK + P) <= m * CH else BLK
                    sta = buf.ap()[:, j * BLK : j * BLK + wcols]
                    last = t.matmul(
                        gacc.ap()[:wcols, :],
                        sta,
                        mov,
                        start=(gi == 0),
                        stop=(gi == total - 1),
                    )
                    gi += 1
            last.then_inc(ten_sem, 1)

        @block.vector
        def _(v: bass.BassEngine):
            v.memset(scr.ap(), 0.03125).then_inc(warm_sem, 1)
            # DVE does the PSUM -> SBUF copy (ACT would pay a ~1.3 us
            # activation-table load for its first ACTIVATE)
            v.wait_ge(ten_sem, 1)
            v.tensor_copy(gsb.ap(), gacc.ap()[:BLK, :]).then_inc(copy_sem, 1)

        @block.scalar
        def _(sc: bass.BassEngine):
            issue_dmas(sc, "act")

        @block.sync
        def _(s: bass.BassEngine):
            issue_dmas(s, "sp")
            s.wait_ge(copy_sem, 1)
            s.dma_start(out=out_g.ap(), in_=gsb.ap()).then_inc(odma_sem, 16)
            s.wait_ge(odma_sem, 16)

    return nc


def pack_inputs(embeddings, source_indicators, npad=NPAD):
    """(B,F,T,D)+(B,F,T,S) -> per-core padded interleaved (npad, 24) fp8."""
    b = embeddings.shape[0]
    n = embeddings.shape[1] * embeddings.shape[2]
    e = np.asarray(embeddings, dtype=np.float32).reshape(b, n, D)
    v = np.asarray(source_indicators, dtype=np.float32).reshape(b, n, S)
    evp = np.zeros((b, npad, CH), dtype=FP8)
    evp[:, :n, :D] = e.astype(FP8)
    evp[:, :n, D : D + S] = v.astype(FP8)
    evp[:, :, D + S] = np.asarray(1.0, dtype=FP8)
    return evp


def reduce_outputs(res):
    """Per-core raw output -> (G_b, EtV_b, colsum_b) in float64.

    Output rows follow the 24-col stationary chunks, output columns the
    23-col (pad-skipped) moving chunks."""
    out_g = np.asarray(res["out_g"], dtype=np.float64)
    g_b = np.zeros((D, D))
    etv_b = np.zeros((D, S))
    colsum_b = np.zeros(S)
    for c in range(C):
        blk = out_g[c * CH : c * CH + CH, c * MCOL : c * MCOL + MCOL]
        g_b += blk[:D, :D]
        etv_b += blk[:D, D : D + S]
        colsum_b += blk[D + S, D : D + S]
    return g_b, etv_b, colsum_b


_NC_CACHE = {}


def _get_nc():
    if "nc" not in _NC_CACHE:
        _NC_CACHE["nc"] = build_bass()
    return _NC_CACHE["nc"]


def kernel(embeddings, source_indicators):
    evp = pack_inputs(embeddings, source_indicators)
    nc = _get_nc()
    in_maps = [{"ev": np.ascontiguousarray(evp[b])} for b in range(B)]
    results = run_bass_kernel_spmd(nc, in_maps, list(range(B))).results

    loss = 0.0
    norms = []
    for b in range(B):
        g_b, etv_b, colsum_b = reduce_outputs(results[b])
        ety = etv_b / (colsum_b[None, :] + 1e-8)
        loss += float(np.sum(ety * ety))
        norms.append(float(np.sum(g_b * g_b)))
    norm_term = float(np.mean(norms))
    return np.float32(-loss / (norm_term + 1e-8))
